# revision 35
# baseline (speedup 1.0000x reference)
"""ALiBi attention (B=2, S=2048, HID=1024, H=16, D=64) on 8 TRN2 NeuronCores.

Sharding: core c -> batch b = c//4, head-group g = c%4 (4 heads = 256 dims).
Each core computes q/k/v projections for its head block, transposed-layout
attention, and a partial output projection; the host sums the 4 partials per
batch and folds the (linear-exact) bv/bo bias terms.

Math trick: softmax_j(qk/8 + slope*(j-i)) row-shifts to exp(qk/8 +
slope*(j-(S-1)) - SHIFT) / sum_j(...), whose additive term depends only on the
key index j. With scores computed transposed (keys on the partition axis),
that term is a per-partition bias folded into the ScalarE exp -- no row-max
pass, no partition-axis reductions. The softmax denominator comes from an
appended ones-column on V; normalization happens on the d=64 ctx rows.
"""

import math
from contextlib import ExitStack

import numpy as np
import ml_dtypes

import concourse.mybir as mybir
import concourse.tile as tile
from concourse import bacc
from concourse.bass_utils import run_bass_kernel_spmd

B, S, HID, H = 2, 2048, 1024, 16
D = 64
NH = 4            # heads per core
DH = NH * D       # 256 dims per core
P = 128
NCORES = 8
SHIFT = 12.0
BF16 = mybir.dt.bfloat16
F32 = mybir.dt.float32

KK = S // P       # 16 key tiles
NQ = 4            # q free chunks of 512
FD = 512

# Head-slot schedule: every core runs 3 full heads + 1 windowed head (last
# WIN_KK key tiles). ALiBi slope * distance makes earlier keys' weights
# < e^-26 relative for heads 0-3, so a 384-key window is exact to ~1e-6.
WIN_KK = 3
SLOT_KK0 = [0, 0, 0, KK - WIN_KK]
# per batch-core (core % 4): global head index for each of the 4 slots
HEADS_OF_BC = [
    [6, 7, 8, 0],
    [9, 10, 11, 1],
    [12, 13, 4, 2],
    [14, 15, 5, 3],
]


def _build(loop_r=1):
    nc = bacc.Bacc("TRN2", target_bir_lowering=False, debug=False)
    xT = nc.declare_dram_parameter("xT", [HID, S], BF16, isOutput=False)
    wq = nc.declare_dram_parameter("wqT", [HID, DH], BF16, isOutput=False)
    wk = nc.declare_dram_parameter("wkT", [HID, DH], BF16, isOutput=False)
    wv = nc.declare_dram_parameter("wvT", [HID, DH], BF16, isOutput=False)
    wo = nc.declare_dram_parameter("woS", [DH, HID], BF16, isOutput=False)
    bp = nc.declare_dram_parameter("bpack", [P, 68], F32, isOutput=False)
    we = nc.declare_dram_parameter("wexp", [P, KK * NH * D], BF16, isOutput=False)
    out = nc.declare_dram_parameter("out", [S, HID], BF16, isOutput=True)

    Exp = mybir.ActivationFunctionType.Exp

    with tile.TileContext(nc) as tc, ExitStack() as ctx:
        if loop_r > 1:
            ctx.enter_context(tc.For_i(0, loop_r, 1))
        persist = ctx.enter_context(tc.tile_pool(name="persist", bufs=1))
        work = ctx.enter_context(tc.tile_pool(name="work", bufs=3))
        pmm = ctx.enter_context(tc.tile_pool(name="pmm", bufs=2, space="PSUM"))
        pacc = ctx.enter_context(tc.tile_pool(name="pacc", bufs=2, space="PSUM"))

        xT_sb = persist.tile([P, 8, S], BF16, tag="xT")
        wq_sb = persist.tile([P, 8, DH], BF16, tag="wq")
        wk_sb = persist.tile([P, 8, DH], BF16, tag="wk")
        wv_sb = persist.tile([P, 8, DH], BF16, tag="wv")
        wo_sb = persist.tile([P, 2, HID], BF16, tag="wo")
        bp_sb = persist.tile([P, 68], F32, tag="bp")
        qT_sb = persist.tile([P, 2, S], BF16, tag="qT")
        kT_sb = persist.tile([P, 2, S], BF16, tag="kT")
        v_sb = persist.tile([P, KK, NH, D + 1], BF16, tag="v")
        ctxT_sb = persist.tile([P, 2, S], BF16, tag="ctxT")
        we_sb = persist.tile([P, KK, NH, D], BF16, tag="wexp")
        ones_sb = persist.tile([1, D], BF16, tag="ones")

        for c in range(8):
            nc.sync.dma_start(xT_sb[:, c, :], xT[c * P:(c + 1) * P, :])
        nc.sync.dma_start(wq_sb[:], wq[:, :].rearrange("(o p) d -> p o d", p=P))
        nc.sync.dma_start(wk_sb[:], wk[:, :].rearrange("(o p) d -> p o d", p=P))
        nc.sync.dma_start(wv_sb[:], wv[:, :].rearrange("(o p) d -> p o d", p=P))
        nc.sync.dma_start(wo_sb[:], wo[:, :].rearrange("(o p) d -> p o d", p=P))
        nc.sync.dma_start(bp_sb[:], bp[:, :])
        nc.sync.dma_start(
            we_sb[:], we[:, :].rearrange("p (k h d) -> p k h d", h=NH, d=D))
        nc.vector.memset(ones_sb[:], 1.0)

        # ---- projection emitters (called interleaved into attention below)
        def qk_chain(w_sb, dst, add_bias, m, n):
            ps = pmm.tile([P, FD], F32, tag="mm")
            for c in range(8):
                nc.tensor.matmul(
                    ps[:],
                    w_sb[:, c, m * P:(m + 1) * P],
                    xT_sb[:, c, n * FD:(n + 1) * FD],
                    start=(c == 0), stop=(c == 7),
                )
            if add_bias:
                nc.vector.tensor_scalar_add(
                    dst[:, m, n * FD:(n + 1) * FD], ps[:],
                    bp_sb[:, 64 + m:65 + m])
            else:
                nc.vector.tensor_copy(
                    dst[:, m, n * FD:(n + 1) * FD], ps[:])

        # v projection, token-major; scaled by the per-key ALiBi weight
        # w_j = exp(slope*(j-(S-1)) - SHIFT) so the ScalarE exp needs no
        # bias. Column D holds w_j itself (softmax-denominator accumulator).
        def v_chain(kk):
            ps = pmm.tile([P, DH], F32, tag="mm")
            for c in range(8):
                nc.tensor.matmul(
                    ps[:],
                    xT_sb[:, c, kk * P:(kk + 1) * P],
                    wv_sb[:, c, :],
                    start=(c == 0), stop=(c == 7),
                )
            nc.vector.tensor_tensor(
                v_sb[:, kk, :, 0:D],
                ps[:, :].rearrange("p (h d) -> p h d", d=D),
                we_sb[:, kk, :, :], mybir.AluOpType.mult)

        for h in range(NH):
            nc.vector.tensor_copy(
                v_sb[:, :, h, D:D + 1],
                bp_sb[:, h * KK:(h + 1) * KK].rearrange("p (k o) -> p k o", o=1))

        # q/k projections for the m=0 dim-half (head slots 0/1) up front;
        # the m=1 half and the v projection are emitted inside the group01
        # attention loop so they fill PE slack under the scalar-bound exp.
        for n in range(NQ):
            qk_chain(wq_sb, qT_sb, True, 0, n)
            qk_chain(wk_sb, kT_sb, False, 0, n)
        m1_chains = [(wq_sb, qT_sb, True), (wk_sb, kT_sb, False)]
        m1_todo = [(w, d, b, n) for (w, d, b) in m1_chains for n in range(NQ)]

        # ---- attention: scoresT -> exp (bias-free) -> PV accumulate.
        # Two heads run per QK step: one lhsT at partition base 0, one at
        # base 64 -> concurrent PE row groups, one shared [128,1024] exp.
        # Slot2's unpaired key tiles self-pair across kk via kdup/qdup
        # (slot2's kT/qT copied to partition base 64).
        kdup = persist.tile([P, S], BF16, tag="kdup")
        qdup = persist.tile([P, S], BF16, tag="qdup")

        def kq_ap(slot, kk, n, dup=False):
            if dup:
                return (kdup[D:P, kk * P:(kk + 1) * P],
                        qdup[D:P, n * FD:(n + 1) * FD])
            po, mc = D * (slot % 2), slot // 2
            return (kT_sb[po:po + D, mc, kk * P:(kk + 1) * P],
                    qT_sb[po:po + D, mc, n * FD:(n + 1) * FD])

        def norm2(slots, n, accs):
            # both slots at once: two reciprocals off the PSUM den rows into
            # one [1, 2FD] tile, one bf16 convert, a pair of PE broadcasts
            # and a mult per slot.
            den2 = work.tile([1, 2 * FD], F32, tag="den", name="den")
            for i, s in enumerate(slots):
                nc.vector.tensor_copy(den2[:, i * FD:(i + 1) * FD],
                                      accs[s][D:D + 1, :])
            recf = work.tile([1, 2 * FD], F32, tag="recf", name="recf")
            nc.vector.reciprocal_approx_fast(out=recf[:], in_=den2[:])
            bcs = work.tile([D, 2 * FD], F32, tag="bcs", name="bcs")
            nc.gpsimd.partition_broadcast(bcs[:], recf[:])
            for i, s in enumerate(slots):
                po, mc = D * (s % 2), s // 2
                nc.vector.tensor_tensor(
                    ctxT_sb[po:po + D, mc, n * FD:(n + 1) * FD],
                    accs[s][0:D, :], bcs[:, i * FD:(i + 1) * FD],
                    mybir.AluOpType.mult)

        # (left, right) work items per n-chunk. Left operands sit at
        # partition base 0 (slots 0/2), right at base 64 (slots 1/3, dups).
        # item = (slot, kk, dup); acc routing and start/stop computed below.
        pair01 = [((0, kk, False), (1, kk, False)) for kk in range(KK)]
        pair23 = ([((2, 2 * i, False), (2, 2 * i + 1, True)) for i in range(6)]
                  + [((2, 12, False), None)]
                  + [((2, kk, False), (3, kk, False))
                     for kk in range(KK - WIN_KK, KK)])

        # ---- output projection (partial over this core's 256 dims)
        def outproj(m):
            ob = work.tile([P, HID], BF16, tag="ob")
            for n2 in range(2):
                ps = pmm.tile([P, FD], F32, tag="mm")
                for c in range(2):
                    nc.tensor.matmul(
                        ps[:],
                        ctxT_sb[:, c, m * P:(m + 1) * P],
                        wo_sb[:, c, n2 * FD:(n2 + 1) * FD],
                        start=(c == 0), stop=(c == 1),
                    )
                if n2 == 0:
                    nc.scalar.copy(ob[:, n2 * FD:(n2 + 1) * FD], ps[:])
                else:
                    nc.vector.tensor_copy(ob[:, n2 * FD:(n2 + 1) * FD], ps[:])
            nc.sync.dma_start(out[m * P:(m + 1) * P, :], ob[:])

        pending = None
        op_todo = []
        m1_idx = 0
        for gi, (pairs, slots) in enumerate(((pair01, (0, 1)),
                                             (pair23, (2, 3)))):
            total = {s: 0 for s in slots}
            for lt, rt in pairs:
                for it in (lt, rt):
                    if it is not None:
                        total[it[0]] += 1
            for n in range(NQ):
                accs = {s: pacc.tile([D + 1, FD], F32, tag=f"acc{s % 2}",
                                     name=f"acc{s}") for s in slots}
                nwr = {s: 0 for s in slots}
                for pi, (lt, rt) in enumerate(pairs):
                    # group01/n0: produce v tile kk=pi just ahead of its PV
                    if gi == 0 and n == 0 and pi < KK:
                        v_chain(pi)
                    wid = FD if rt is None else 2 * FD
                    st = pmm.tile([P, 2 * FD], F32, tag="mm", name="st")
                    pt = work.tile([P, 2 * FD], BF16, tag="pt", name="pt")
                    for side, it in enumerate((lt, rt)):
                        if it is None:
                            continue
                        slot, kk, dup = it
                        kap, qap = kq_ap(slot, kk, n, dup)
                        nc.tensor.matmul(st[:, side * FD:(side + 1) * FD],
                                         kap, qap, start=True, stop=True)
                    nc.scalar.activation(pt[:, 0:wid], st[:, 0:wid], Exp,
                                         bias=0.0, scale=1.0)
                    for side, it in enumerate((lt, rt)):
                        if it is None:
                            continue
                        slot, kk, dup = it
                        nc.tensor.matmul(
                            accs[slot][:], v_sb[:, kk, slot, :],
                            pt[:, side * FD:(side + 1) * FD],
                            start=(nwr[slot] == 0),
                            stop=(nwr[slot] == total[slot] - 1),
                        )
                        nwr[slot] += 1
                    # previous chunk's norm, emitted after this chunk's
                    # first pair so its PE/DVE work hides under the
                    # already-running QK->exp->PV pipeline
                    if pi == 0 and pending is not None:
                        prev, pending = pending, None
                        norm2(*prev)
                        if prev[0] == (2, 3):
                            op_todo += [4 * prev[1] + i for i in range(4)]
                    # group01/n1-3: m=1 q/k projection chains in PE slack
                    if (gi == 0 and n >= 1 and pi in (1, 6, 11)
                            and m1_idx < len(m1_todo)):
                        w, dst, ab, nn = m1_todo[m1_idx]
                        m1_idx += 1
                        qk_chain(w, dst, ab, 1, nn)
                    # group23: drain ready output-projection tiles
                    if gi == 1 and op_todo and pi >= 1:
                        outproj(op_todo.pop(0))
                pending = (slots, n, accs)
            if gi == 0:
                nc.vector.tensor_copy(kdup[D:P, :], kT_sb[0:D, 1, :])
                nc.vector.tensor_copy(qdup[D:P, :], qT_sb[0:D, 1, :])
        norm2(*pending)
        op_todo += [12, 13, 14, 15]
        for m in op_todo:
            outproj(m)

    nc.compile()
    return nc


_nc_cache = None


def _in_map_for_core(c, x, Wq, bq, Wk, Wv, Wo, slopes):
    b, g = c // 4, c % 4
    heads = HEADS_OF_BC[g]
    rows = np.concatenate([np.arange(h * D, (h + 1) * D) for h in heads])
    bf = ml_dtypes.bfloat16
    xTc = np.ascontiguousarray(x[b].T).astype(bf)
    wqT = np.ascontiguousarray(Wq[rows].T * 0.125).astype(bf)
    wkT = np.ascontiguousarray(Wk[rows].T).astype(bf)
    wvT = np.ascontiguousarray(Wv[rows].T).astype(bf)
    woS = np.ascontiguousarray(Wo[:, rows].T).astype(bf)
    bp = np.zeros((P, 68), np.float32)
    j = np.arange(P, dtype=np.float32)
    for hh in range(NH):
        sl = float(slopes[heads[hh]])
        for kk in range(KK):
            # ALiBi weight w_j = exp(slope*(j-(S-1)) - SHIFT), folded into V
            bp[:, hh * KK + kk] = np.exp(sl * (kk * P + j - (S - 1)) - SHIFT)
    bqs = bq[rows].astype(np.float32) * 0.125
    bp[:, 64] = bqs[0:P]
    bp[:, 65] = bqs[P:2 * P]
    # wexp[j, kk, h, d] = w_j for (head h, key tile kk), replicated over d
    wex = np.broadcast_to(
        bp[:, :64].reshape(P, NH, KK).transpose(0, 2, 1)[:, :, :, None],
        (P, KK, NH, D)).reshape(P, KK * NH * D).astype(bf)
    return {"xT": xTc, "wqT": wqT, "wkT": wkT, "wvT": wvT, "woS": woS,
            "bpack": bp, "wexp": np.ascontiguousarray(wex)}


def kernel(x, Wq, bq, Wk, bk, Wv, bv, Wo, bo, slopes):
    global _nc_cache
    x = np.asarray(x, np.float32)
    Wq = np.asarray(Wq, np.float32)
    Wk = np.asarray(Wk, np.float32)
    Wv = np.asarray(Wv, np.float32)
    Wo = np.asarray(Wo, np.float32)
    bq = np.asarray(bq, np.float32)
    bv = np.asarray(bv, np.float32)
    bo = np.asarray(bo, np.float32)
    slopes = np.asarray(slopes, np.float32)

    if _nc_cache is None:
        _nc_cache = _build()
    nc = _nc_cache

    in_maps = [_in_map_for_core(c, x, Wq, bq, Wk, Wv, Wo, slopes)
               for c in range(NCORES)]
    res = run_bass_kernel_spmd(nc, in_maps, core_ids=list(range(NCORES)))
    global LAST_RESULT
    LAST_RESULT = res

    # bk shifts every score in a row i by q_i . bk (constant over j) -> cancels
    # in softmax. bv/bo are linear post-attention terms, folded here exactly.
    bias_term = (bv @ Wo.T + bo)[None, :]
    full = np.zeros((B, S, HID), np.float32)
    for b in range(B):
        acc = np.zeros((S, HID), np.float32)
        for g in range(4):
            acc += np.asarray(res.results[b * 4 + g]["out"]).astype(np.float32)
        full[b] = acc + bias_term
    return full



# revision 40
# speedup vs baseline: 1.1239x; 1.1239x over previous
"""ALiBi attention (B=2, S=2048, HID=1024, H=16, D=64) on 8 TRN2 NeuronCores.

Sharding: core c -> batch b = c//4, head-group g = c%4 (4 heads = 256 dims).
Each core computes q/k/v projections for its head block, transposed-layout
attention, and a partial output projection; the host sums the 4 partials per
batch and folds the (linear-exact) bv/bo bias terms.

Math trick: softmax_j(qk/8 + slope*(j-i)) row-shifts to exp(qk/8 +
slope*(j-(S-1)) - SHIFT) / sum_j(...), whose additive term depends only on the
key index j. With scores computed transposed (keys on the partition axis),
that term is a per-partition bias folded into the ScalarE exp -- no row-max
pass, no partition-axis reductions. The softmax denominator comes from an
appended ones-column on V; normalization happens on the d=64 ctx rows.
"""

import math
from contextlib import ExitStack

import numpy as np
import ml_dtypes

import concourse.mybir as mybir
import concourse.tile as tile
from concourse import bacc
from concourse.bass_utils import run_bass_kernel_spmd

B, S, HID, H = 2, 2048, 1024, 16
D = 64
NH = 4            # heads per core
DH = NH * D       # 256 dims per core
P = 128
NCORES = 8
SHIFT = 12.0
BF16 = mybir.dt.bfloat16
F32 = mybir.dt.float32

KK = S // P       # 16 key tiles
NQ = 4            # q free chunks of 512
FD = 512

# Head-slot schedule: every core runs 3 full heads + 1 windowed head (last
# WIN_KK key tiles). ALiBi slope * distance makes earlier keys' weights
# < e^-26 relative for heads 0-3, so a 384-key window is exact to ~1e-6.
WIN_KK = 3
SLOT_KK0 = [0, 0, 0, KK - WIN_KK]
# per batch-core (core % 4): global head index for each of the 4 slots
HEADS_OF_BC = [
    [6, 7, 8, 0],
    [9, 10, 11, 1],
    [12, 13, 4, 2],
    [14, 15, 5, 3],
]


def _build(loop_r=1):
    nc = bacc.Bacc("TRN2", target_bir_lowering=False, debug=False)
    xT = nc.declare_dram_parameter("xT", [HID, S], BF16, isOutput=False)
    wq = nc.declare_dram_parameter("wqT", [HID, DH], BF16, isOutput=False)
    wk = nc.declare_dram_parameter("wkT", [HID, DH], BF16, isOutput=False)
    wv = nc.declare_dram_parameter("wvT", [HID, DH], BF16, isOutput=False)
    wo = nc.declare_dram_parameter("woS", [DH, HID], BF16, isOutput=False)
    bp = nc.declare_dram_parameter("bpack", [P, 68], F32, isOutput=False)
    we = nc.declare_dram_parameter("wexp", [P, KK * NH * D], BF16, isOutput=False)
    out = nc.declare_dram_parameter("out", [S, HID], BF16, isOutput=True)

    Exp = mybir.ActivationFunctionType.Exp

    with tile.TileContext(nc) as tc, ExitStack() as ctx:
        if loop_r > 1:
            ctx.enter_context(tc.For_i(0, loop_r, 1))
        persist = ctx.enter_context(tc.tile_pool(name="persist", bufs=1))
        work = ctx.enter_context(tc.tile_pool(name="work", bufs=3))
        pmm = ctx.enter_context(tc.tile_pool(name="pmm", bufs=2, space="PSUM"))
        pacc = ctx.enter_context(tc.tile_pool(name="pacc", bufs=2, space="PSUM"))

        xT_sb = persist.tile([P, 8, S], BF16, tag="xT")
        wq_sb = persist.tile([P, 8, DH], BF16, tag="wq")
        wk_sb = persist.tile([P, 8, DH], BF16, tag="wk")
        wv_sb = persist.tile([P, 8, DH], BF16, tag="wv")
        wo_sb = persist.tile([P, 2, HID], BF16, tag="wo")
        bp_sb = persist.tile([P, 68], F32, tag="bp")
        qT_sb = persist.tile([P, 2, S], BF16, tag="qT")
        kT_sb = persist.tile([P, 2, S], BF16, tag="kT")
        v_sb = persist.tile([P, KK, NH, D + 1], BF16, tag="v")
        ctxT_sb = persist.tile([P, 2, S], BF16, tag="ctxT")
        we_sb = persist.tile([P, KK, NH, D], BF16, tag="wexp")
        ones_sb = persist.tile([1, D], BF16, tag="ones")

        for c in range(8):
            nc.sync.dma_start(xT_sb[:, c, :], xT[c * P:(c + 1) * P, :])
        nc.sync.dma_start(wq_sb[:], wq[:, :].rearrange("(o p) d -> p o d", p=P))
        nc.sync.dma_start(wk_sb[:], wk[:, :].rearrange("(o p) d -> p o d", p=P))
        nc.sync.dma_start(wv_sb[:], wv[:, :].rearrange("(o p) d -> p o d", p=P))
        nc.sync.dma_start(wo_sb[:], wo[:, :].rearrange("(o p) d -> p o d", p=P))
        nc.sync.dma_start(bp_sb[:], bp[:, :])
        nc.sync.dma_start(
            we_sb[:], we[:, :].rearrange("p (k h d) -> p k h d", h=NH, d=D))
        nc.vector.memset(ones_sb[:], 1.0)

        # ---- projection emitters (called interleaved into attention below)
        def qk_chain(w_sb, dst, add_bias, m, n):
            ps = pmm.tile([P, FD], F32, tag="mm")
            for c in range(8):
                nc.tensor.matmul(
                    ps[:],
                    w_sb[:, c, m * P:(m + 1) * P],
                    xT_sb[:, c, n * FD:(n + 1) * FD],
                    start=(c == 0), stop=(c == 7),
                )
            if add_bias:
                nc.vector.tensor_scalar_add(
                    dst[:, m, n * FD:(n + 1) * FD], ps[:],
                    bp_sb[:, 64 + m:65 + m])
            else:
                nc.vector.tensor_copy(
                    dst[:, m, n * FD:(n + 1) * FD], ps[:])

        # v projection, token-major; scaled by the per-key ALiBi weight
        # w_j = exp(slope*(j-(S-1)) - SHIFT) so the ScalarE exp needs no
        # bias. Column D holds w_j itself (softmax-denominator accumulator).
        def v_chain(kk):
            ps = pmm.tile([P, DH], F32, tag="mm")
            for c in range(8):
                nc.tensor.matmul(
                    ps[:],
                    xT_sb[:, c, kk * P:(kk + 1) * P],
                    wv_sb[:, c, :],
                    start=(c == 0), stop=(c == 7),
                )
            nc.vector.tensor_tensor(
                v_sb[:, kk, :, 0:D],
                ps[:, :].rearrange("p (h d) -> p h d", d=D),
                we_sb[:, kk, :, :], mybir.AluOpType.mult)

        for h in range(NH):
            nc.vector.tensor_copy(
                v_sb[:, :, h, D:D + 1],
                bp_sb[:, h * KK:(h + 1) * KK].rearrange("p (k o) -> p k o", o=1))

        # ---- attention: scoresT -> exp (bias-free) -> PV accumulate.
        # Two heads run per QK step: one lhsT at partition base 0, one at
        # base 64 -> concurrent PE row groups, one shared [128,1024] exp.
        # Slot2's unpaired key tiles self-pair across kk via kdup/qdup
        # (slot2's kT/qT copied to partition base 64).
        kdup = persist.tile([P, S], BF16, tag="kdup")
        qdup = persist.tile([P, S], BF16, tag="qdup")

        # all q/k projections up front (attention needs qT/kT); the v
        # projection chains are emitted inside group01 chunk 0 where the
        # scalar-bound exp cadence leaves PE slack.
        for m in range(2):
            for n in range(NQ):
                qk_chain(wq_sb, qT_sb, True, m, n)
                qk_chain(wk_sb, kT_sb, False, m, n)
        nc.vector.tensor_copy(kdup[D:P, :], kT_sb[0:D, 1, :])
        nc.vector.tensor_copy(qdup[D:P, :], qT_sb[0:D, 1, :])

        def kq_ap(slot, kk, n, dup=False):
            if dup:
                return (kdup[D:P, kk * P:(kk + 1) * P],
                        qdup[D:P, n * FD:(n + 1) * FD])
            po, mc = D * (slot % 2), slot // 2
            return (kT_sb[po:po + D, mc, kk * P:(kk + 1) * P],
                    qT_sb[po:po + D, mc, n * FD:(n + 1) * FD])

        def norm2(slots, n, accs):
            # both slots at once: two reciprocals off the PSUM den rows into
            # one [1, 2FD] tile, one bf16 convert, a pair of PE broadcasts
            # and a mult per slot.
            den2 = work.tile([1, 2 * FD], F32, tag="den", name="den")
            for i, s in enumerate(slots):
                nc.vector.tensor_copy(den2[:, i * FD:(i + 1) * FD],
                                      accs[s][D:D + 1, :])
            recf = work.tile([1, 2 * FD], F32, tag="recf", name="recf")
            nc.vector.reciprocal_approx_fast(out=recf[:], in_=den2[:])
            bcs = work.tile([D, 2 * FD], F32, tag="bcs", name="bcs")
            nc.gpsimd.partition_broadcast(bcs[:], recf[:])
            for i, s in enumerate(slots):
                po, mc = D * (s % 2), s // 2
                nc.vector.tensor_tensor(
                    ctxT_sb[po:po + D, mc, n * FD:(n + 1) * FD],
                    accs[s][0:D, :], bcs[:, i * FD:(i + 1) * FD],
                    mybir.AluOpType.mult)

        # (left, right) work items per n-chunk. Left operands sit at
        # partition base 0 (slots 0/2), right at base 64 (slots 1/3, dups).
        # item = (slot, kk, dup); acc routing and start/stop computed below.
        pair01 = [((0, kk, False), (1, kk, False)) for kk in range(KK)]
        pair23 = ([((2, 2 * i, False), (2, 2 * i + 1, True)) for i in range(6)]
                  + [((2, 12, False), None)]
                  + [((2, kk, False), (3, kk, False))
                     for kk in range(KK - WIN_KK, KK)])

        # ---- output projection (partial over this core's 256 dims)
        def outproj(m):
            ob = work.tile([P, HID], BF16, tag="ob")
            for n2 in range(2):
                ps = pmm.tile([P, FD], F32, tag="mm")
                for c in range(2):
                    nc.tensor.matmul(
                        ps[:],
                        ctxT_sb[:, c, m * P:(m + 1) * P],
                        wo_sb[:, c, n2 * FD:(n2 + 1) * FD],
                        start=(c == 0), stop=(c == 1),
                    )
                if n2 == 0:
                    nc.scalar.copy(ob[:, n2 * FD:(n2 + 1) * FD], ps[:])
                else:
                    nc.vector.tensor_copy(ob[:, n2 * FD:(n2 + 1) * FD], ps[:])
            nc.sync.dma_start(out[m * P:(m + 1) * P, :], ob[:])

        pending = None
        for gi, (pairs, slots) in enumerate(((pair01, (0, 1)),
                                             (pair23, (2, 3)))):
            total = {s: 0 for s in slots}
            for lt, rt in pairs:
                for it in (lt, rt):
                    if it is not None:
                        total[it[0]] += 1
            for n in range(NQ):
                accs = {s: pacc.tile([D + 1, FD], F32, tag=f"acc{s % 2}",
                                     name=f"acc{s}") for s in slots}
                nwr = {s: 0 for s in slots}
                for pi, (lt, rt) in enumerate(pairs):
                    # group01/n0: produce v tile kk=pi just ahead of its PV
                    if gi == 0 and n == 0 and pi < KK:
                        v_chain(pi)
                    wid = FD if rt is None else 2 * FD
                    st = pmm.tile([P, 2 * FD], F32, tag="mm", name="st")
                    pt = work.tile([P, 2 * FD], BF16, tag="pt", name="pt")
                    for side, it in enumerate((lt, rt)):
                        if it is None:
                            continue
                        slot, kk, dup = it
                        kap, qap = kq_ap(slot, kk, n, dup)
                        nc.tensor.matmul(st[:, side * FD:(side + 1) * FD],
                                         kap, qap, start=True, stop=True)
                    nc.scalar.activation(pt[:, 0:wid], st[:, 0:wid], Exp,
                                         bias=0.0, scale=1.0)
                    for side, it in enumerate((lt, rt)):
                        if it is None:
                            continue
                        slot, kk, dup = it
                        nc.tensor.matmul(
                            accs[slot][:], v_sb[:, kk, slot, :],
                            pt[:, side * FD:(side + 1) * FD],
                            start=(nwr[slot] == 0),
                            stop=(nwr[slot] == total[slot] - 1),
                        )
                        nwr[slot] += 1
                    # previous chunk's norm, emitted after this chunk's
                    # first pair so its PE/DVE work hides under the
                    # already-running QK->exp->PV pipeline
                    if pi == 0 and pending is not None:
                        prev, pending = pending, None
                        norm2(*prev)
                pending = (slots, n, accs)
        norm2(*pending)
        for m in range(KK):
            outproj(m)

    nc.compile()
    return nc


_nc_cache = None


def _in_map_for_core(c, x, Wq, bq, Wk, Wv, Wo, slopes):
    b, g = c // 4, c % 4
    heads = HEADS_OF_BC[g]
    rows = np.concatenate([np.arange(h * D, (h + 1) * D) for h in heads])
    bf = ml_dtypes.bfloat16
    xTc = np.ascontiguousarray(x[b].T).astype(bf)
    wqT = np.ascontiguousarray(Wq[rows].T * 0.125).astype(bf)
    wkT = np.ascontiguousarray(Wk[rows].T).astype(bf)
    wvT = np.ascontiguousarray(Wv[rows].T).astype(bf)
    woS = np.ascontiguousarray(Wo[:, rows].T).astype(bf)
    bp = np.zeros((P, 68), np.float32)
    j = np.arange(P, dtype=np.float32)
    for hh in range(NH):
        sl = float(slopes[heads[hh]])
        for kk in range(KK):
            # ALiBi weight w_j = exp(slope*(j-(S-1)) - SHIFT), folded into V
            bp[:, hh * KK + kk] = np.exp(sl * (kk * P + j - (S - 1)) - SHIFT)
    bqs = bq[rows].astype(np.float32) * 0.125
    bp[:, 64] = bqs[0:P]
    bp[:, 65] = bqs[P:2 * P]
    # wexp[j, kk, h, d] = w_j for (head h, key tile kk), replicated over d
    wex = np.broadcast_to(
        bp[:, :64].reshape(P, NH, KK).transpose(0, 2, 1)[:, :, :, None],
        (P, KK, NH, D)).reshape(P, KK * NH * D).astype(bf)
    return {"xT": xTc, "wqT": wqT, "wkT": wkT, "wvT": wvT, "woS": woS,
            "bpack": bp, "wexp": np.ascontiguousarray(wex)}


def kernel(x, Wq, bq, Wk, bk, Wv, bv, Wo, bo, slopes):
    global _nc_cache
    x = np.asarray(x, np.float32)
    Wq = np.asarray(Wq, np.float32)
    Wk = np.asarray(Wk, np.float32)
    Wv = np.asarray(Wv, np.float32)
    Wo = np.asarray(Wo, np.float32)
    bq = np.asarray(bq, np.float32)
    bv = np.asarray(bv, np.float32)
    bo = np.asarray(bo, np.float32)
    slopes = np.asarray(slopes, np.float32)

    if _nc_cache is None:
        _nc_cache = _build()
    nc = _nc_cache

    in_maps = [_in_map_for_core(c, x, Wq, bq, Wk, Wv, Wo, slopes)
               for c in range(NCORES)]
    res = run_bass_kernel_spmd(nc, in_maps, core_ids=list(range(NCORES)))
    global LAST_RESULT
    LAST_RESULT = res

    # bk shifts every score in a row i by q_i . bk (constant over j) -> cancels
    # in softmax. bv/bo are linear post-attention terms, folded here exactly.
    bias_term = (bv @ Wo.T + bo)[None, :]
    full = np.zeros((B, S, HID), np.float32)
    for b in range(B):
        acc = np.zeros((S, HID), np.float32)
        for g in range(4):
            acc += np.asarray(res.results[b * 4 + g]["out"]).astype(np.float32)
        full[b] = acc + bias_term
    return full



# revision 42
# speedup vs baseline: 1.1398x; 1.0142x over previous
"""ALiBi attention (B=2, S=2048, HID=1024, H=16, D=64) on 8 TRN2 NeuronCores.

Sharding: core c -> batch b = c//4, head-group g = c%4 (4 heads = 256 dims).
Each core computes q/k/v projections for its head block, transposed-layout
attention, and a partial output projection; the host sums the 4 partials per
batch and folds the (linear-exact) bv/bo bias terms.

Math trick: softmax_j(qk/8 + slope*(j-i)) row-shifts to exp(qk/8 +
slope*(j-(S-1)) - SHIFT) / sum_j(...), whose additive term depends only on the
key index j. With scores computed transposed (keys on the partition axis),
that term is a per-partition bias folded into the ScalarE exp -- no row-max
pass, no partition-axis reductions. The softmax denominator comes from an
appended ones-column on V; normalization happens on the d=64 ctx rows.
"""

import math
from contextlib import ExitStack

import numpy as np
import ml_dtypes

import concourse.mybir as mybir
import concourse.tile as tile
from concourse import bacc
from concourse.bass_utils import run_bass_kernel_spmd

B, S, HID, H = 2, 2048, 1024, 16
D = 64
NH = 4            # heads per core
DH = NH * D       # 256 dims per core
P = 128
NCORES = 8
SHIFT = 12.0
BF16 = mybir.dt.bfloat16
F32 = mybir.dt.float32

KK = S // P       # 16 key tiles
NQ = 4            # q free chunks of 512
FD = 512

# Head-slot schedule: every core runs 3 full heads + 1 windowed head (last
# WIN_KK key tiles). ALiBi slope * distance makes earlier keys' weights
# < e^-26 relative for heads 0-3, so a 384-key window is exact to ~1e-6.
WIN_KK = 3
SLOT_KK0 = [0, 0, 0, KK - WIN_KK]
# per batch-core (core % 4): global head index for each of the 4 slots
HEADS_OF_BC = [
    [6, 7, 8, 0],
    [9, 10, 11, 1],
    [12, 13, 4, 2],
    [14, 15, 5, 3],
]


def _build(loop_r=1):
    nc = bacc.Bacc("TRN2", target_bir_lowering=False, debug=False)
    xT = nc.declare_dram_parameter("xT", [HID, S], BF16, isOutput=False)
    wq = nc.declare_dram_parameter("wqT", [HID, DH], BF16, isOutput=False)
    wk = nc.declare_dram_parameter("wkT", [HID, DH], BF16, isOutput=False)
    wv = nc.declare_dram_parameter("wvT", [HID, DH], BF16, isOutput=False)
    wo = nc.declare_dram_parameter("woS", [DH, HID], BF16, isOutput=False)
    bp = nc.declare_dram_parameter("bpack", [P, 68], F32, isOutput=False)
    we = nc.declare_dram_parameter("wexp", [P, KK * NH * D], BF16, isOutput=False)
    out = nc.declare_dram_parameter("out", [S, HID], BF16, isOutput=True)

    Exp = mybir.ActivationFunctionType.Exp

    with tile.TileContext(nc) as tc, ExitStack() as ctx:
        if loop_r > 1:
            ctx.enter_context(tc.For_i(0, loop_r, 1))
        persist = ctx.enter_context(tc.tile_pool(name="persist", bufs=1))
        work = ctx.enter_context(tc.tile_pool(name="work", bufs=3))
        pmm = ctx.enter_context(tc.tile_pool(name="pmm", bufs=2, space="PSUM"))
        pacc = ctx.enter_context(tc.tile_pool(name="pacc", bufs=2, space="PSUM"))

        xT_sb = persist.tile([P, 8, S], BF16, tag="xT")
        wq_sb = persist.tile([P, 8, DH], BF16, tag="wq")
        wk_sb = persist.tile([P, 8, DH], BF16, tag="wk")
        wv_sb = persist.tile([P, 8, DH], BF16, tag="wv")
        wo_sb = persist.tile([P, 2, HID], BF16, tag="wo")
        bp_sb = persist.tile([P, 68], F32, tag="bp")
        qT_sb = persist.tile([P, 2, S], BF16, tag="qT")
        kT_sb = persist.tile([P, 2, S], BF16, tag="kT")
        v_sb = persist.tile([P, KK, NH, D + 1], BF16, tag="v")
        ctxT_sb = persist.tile([P, 2, S], BF16, tag="ctxT")
        we_sb = persist.tile([P, KK, NH, D], BF16, tag="wexp")
        ones_sb = persist.tile([1, D], BF16, tag="ones")

        for c in range(8):
            nc.sync.dma_start(xT_sb[:, c, :], xT[c * P:(c + 1) * P, :])
        nc.sync.dma_start(wq_sb[:], wq[:, :].rearrange("(o p) d -> p o d", p=P))
        nc.sync.dma_start(wk_sb[:], wk[:, :].rearrange("(o p) d -> p o d", p=P))
        nc.sync.dma_start(wv_sb[:], wv[:, :].rearrange("(o p) d -> p o d", p=P))
        nc.sync.dma_start(wo_sb[:], wo[:, :].rearrange("(o p) d -> p o d", p=P))
        nc.sync.dma_start(bp_sb[:], bp[:, :])
        nc.sync.dma_start(
            we_sb[:], we[:, :].rearrange("p (k h d) -> p k h d", h=NH, d=D))
        nc.vector.memset(ones_sb[:], 1.0)

        # ---- projection emitters (called interleaved into attention below)
        def qk_chain(w_sb, dst, add_bias, m, n):
            ps = pmm.tile([P, FD], F32, tag="mm")
            for c in range(8):
                nc.tensor.matmul(
                    ps[:],
                    w_sb[:, c, m * P:(m + 1) * P],
                    xT_sb[:, c, n * FD:(n + 1) * FD],
                    start=(c == 0), stop=(c == 7),
                )
            if add_bias:
                nc.vector.tensor_scalar_add(
                    dst[:, m, n * FD:(n + 1) * FD], ps[:],
                    bp_sb[:, 64 + m:65 + m])
            else:
                nc.vector.tensor_copy(
                    dst[:, m, n * FD:(n + 1) * FD], ps[:])

        # v projection, token-major; scaled by the per-key ALiBi weight
        # w_j = exp(slope*(j-(S-1)) - SHIFT) so the ScalarE exp needs no
        # bias. Column D holds w_j itself (softmax-denominator accumulator).
        def v_chain(kk):
            ps = pmm.tile([P, DH], F32, tag="mm")
            for c in range(8):
                nc.tensor.matmul(
                    ps[:],
                    xT_sb[:, c, kk * P:(kk + 1) * P],
                    wv_sb[:, c, :],
                    start=(c == 0), stop=(c == 7),
                )
            nc.vector.tensor_tensor(
                v_sb[:, kk, :, 0:D],
                ps[:, :].rearrange("p (h d) -> p h d", d=D),
                we_sb[:, kk, :, :], mybir.AluOpType.mult)

        for h in range(NH):
            nc.vector.tensor_copy(
                v_sb[:, :, h, D:D + 1],
                bp_sb[:, h * KK:(h + 1) * KK].rearrange("p (k o) -> p k o", o=1))

        # ---- attention: scoresT -> exp (bias-free) -> PV accumulate.
        # Two heads run per QK step: one lhsT at partition base 0, one at
        # base 64 -> concurrent PE row groups, one shared [128,1024] exp.
        # Slot2's unpaired key tiles self-pair across kk via kdup/qdup
        # (slot2's kT/qT copied to partition base 64).
        kdup = persist.tile([P, S], BF16, tag="kdup")
        qdup = persist.tile([P, S], BF16, tag="qdup")

        # all q/k projections up front (attention needs qT/kT); the v
        # projection chains are emitted inside group01 chunk 0 where the
        # scalar-bound exp cadence leaves PE slack.
        for m in range(2):
            for n in range(NQ):
                qk_chain(wq_sb, qT_sb, True, m, n)
                qk_chain(wk_sb, kT_sb, False, m, n)
        nc.vector.tensor_copy(kdup[D:P, :], kT_sb[0:D, 1, :])
        nc.vector.tensor_copy(qdup[D:P, :], qT_sb[0:D, 1, :])
        for kk in range(KK):
            v_chain(kk)

        def kq_ap(slot, kk, n, dup=False):
            if dup:
                return (kdup[D:P, kk * P:(kk + 1) * P],
                        qdup[D:P, n * FD:(n + 1) * FD])
            po, mc = D * (slot % 2), slot // 2
            return (kT_sb[po:po + D, mc, kk * P:(kk + 1) * P],
                    qT_sb[po:po + D, mc, n * FD:(n + 1) * FD])

        def norm2(slots, n, accs):
            # both slots at once: two reciprocals off the PSUM den rows into
            # one [1, 2FD] tile, one bf16 convert, a pair of PE broadcasts
            # and a mult per slot.
            den2 = work.tile([1, 2 * FD], F32, tag="den", name="den")
            for i, s in enumerate(slots):
                nc.vector.tensor_copy(den2[:, i * FD:(i + 1) * FD],
                                      accs[s][D:D + 1, :])
            recf = work.tile([1, 2 * FD], F32, tag="recf", name="recf")
            nc.vector.reciprocal_approx_fast(out=recf[:], in_=den2[:])
            bcs = work.tile([D, 2 * FD], F32, tag="bcs", name="bcs")
            nc.gpsimd.partition_broadcast(bcs[:], recf[:])
            for i, s in enumerate(slots):
                po, mc = D * (s % 2), s // 2
                nc.vector.tensor_tensor(
                    ctxT_sb[po:po + D, mc, n * FD:(n + 1) * FD],
                    accs[s][0:D, :], bcs[:, i * FD:(i + 1) * FD],
                    mybir.AluOpType.mult)

        # (left, right) work items per n-chunk. Left operands sit at
        # partition base 0 (slots 0/2), right at base 64 (slots 1/3, dups).
        # item = (slot, kk, dup); acc routing and start/stop computed below.
        pair01 = [((0, kk, False), (1, kk, False)) for kk in range(KK)]
        pair23 = ([((2, 2 * i, False), (2, 2 * i + 1, True)) for i in range(6)]
                  + [((2, 12, False), None)]
                  + [((2, kk, False), (3, kk, False))
                     for kk in range(KK - WIN_KK, KK)])

        # ---- output projection (partial over this core's 256 dims)
        def outproj(m):
            ob = work.tile([P, HID], BF16, tag="ob")
            for n2 in range(2):
                ps = pmm.tile([P, FD], F32, tag="mm")
                for c in range(2):
                    nc.tensor.matmul(
                        ps[:],
                        ctxT_sb[:, c, m * P:(m + 1) * P],
                        wo_sb[:, c, n2 * FD:(n2 + 1) * FD],
                        start=(c == 0), stop=(c == 1),
                    )
                if n2 == 0:
                    nc.scalar.copy(ob[:, n2 * FD:(n2 + 1) * FD], ps[:])
                else:
                    nc.vector.tensor_copy(ob[:, n2 * FD:(n2 + 1) * FD], ps[:])
            nc.sync.dma_start(out[m * P:(m + 1) * P, :], ob[:])

        pending = None
        for gi, (pairs, slots) in enumerate(((pair01, (0, 1)),
                                             (pair23, (2, 3)))):
            total = {s: 0 for s in slots}
            for lt, rt in pairs:
                for it in (lt, rt):
                    if it is not None:
                        total[it[0]] += 1
            for n in range(NQ):
                accs = {s: pacc.tile([D + 1, FD], F32, tag=f"acc{s % 2}",
                                     name=f"acc{s}") for s in slots}
                nwr = {s: 0 for s in slots}
                for pi, (lt, rt) in enumerate(pairs):
                    wid = FD if rt is None else 2 * FD
                    st = pmm.tile([P, 2 * FD], F32, tag="mm", name="st")
                    pt = work.tile([P, 2 * FD], BF16, tag="pt", name="pt")
                    for side, it in enumerate((lt, rt)):
                        if it is None:
                            continue
                        slot, kk, dup = it
                        kap, qap = kq_ap(slot, kk, n, dup)
                        nc.tensor.matmul(st[:, side * FD:(side + 1) * FD],
                                         kap, qap, start=True, stop=True)
                    nc.scalar.activation(pt[:, 0:wid], st[:, 0:wid], Exp,
                                         bias=0.0, scale=1.0)
                    for side, it in enumerate((lt, rt)):
                        if it is None:
                            continue
                        slot, kk, dup = it
                        nc.tensor.matmul(
                            accs[slot][:], v_sb[:, kk, slot, :],
                            pt[:, side * FD:(side + 1) * FD],
                            start=(nwr[slot] == 0),
                            stop=(nwr[slot] == total[slot] - 1),
                        )
                        nwr[slot] += 1
                    # previous chunk's norm, emitted after this chunk's
                    # first pair so its PE/DVE work hides under the
                    # already-running QK->exp->PV pipeline
                    if pi == 0 and pending is not None:
                        prev, pending = pending, None
                        norm2(*prev)
                pending = (slots, n, accs)
        norm2(*pending)
        for m in range(KK):
            outproj(m)

    nc.compile()
    return nc


_nc_cache = None


def _in_map_for_core(c, x, Wq, bq, Wk, Wv, Wo, slopes):
    b, g = c // 4, c % 4
    heads = HEADS_OF_BC[g]
    rows = np.concatenate([np.arange(h * D, (h + 1) * D) for h in heads])
    bf = ml_dtypes.bfloat16
    xTc = np.ascontiguousarray(x[b].T).astype(bf)
    wqT = np.ascontiguousarray(Wq[rows].T * 0.125).astype(bf)
    wkT = np.ascontiguousarray(Wk[rows].T).astype(bf)
    wvT = np.ascontiguousarray(Wv[rows].T).astype(bf)
    woS = np.ascontiguousarray(Wo[:, rows].T).astype(bf)
    bp = np.zeros((P, 68), np.float32)
    j = np.arange(P, dtype=np.float32)
    for hh in range(NH):
        sl = float(slopes[heads[hh]])
        for kk in range(KK):
            # ALiBi weight w_j = exp(slope*(j-(S-1)) - SHIFT), folded into V
            bp[:, hh * KK + kk] = np.exp(sl * (kk * P + j - (S - 1)) - SHIFT)
    bqs = bq[rows].astype(np.float32) * 0.125
    bp[:, 64] = bqs[0:P]
    bp[:, 65] = bqs[P:2 * P]
    # wexp[j, kk, h, d] = w_j for (head h, key tile kk), replicated over d
    wex = np.broadcast_to(
        bp[:, :64].reshape(P, NH, KK).transpose(0, 2, 1)[:, :, :, None],
        (P, KK, NH, D)).reshape(P, KK * NH * D).astype(bf)
    return {"xT": xTc, "wqT": wqT, "wkT": wkT, "wvT": wvT, "woS": woS,
            "bpack": bp, "wexp": np.ascontiguousarray(wex)}


def kernel(x, Wq, bq, Wk, bk, Wv, bv, Wo, bo, slopes):
    global _nc_cache
    x = np.asarray(x, np.float32)
    Wq = np.asarray(Wq, np.float32)
    Wk = np.asarray(Wk, np.float32)
    Wv = np.asarray(Wv, np.float32)
    Wo = np.asarray(Wo, np.float32)
    bq = np.asarray(bq, np.float32)
    bv = np.asarray(bv, np.float32)
    bo = np.asarray(bo, np.float32)
    slopes = np.asarray(slopes, np.float32)

    if _nc_cache is None:
        _nc_cache = _build()
    nc = _nc_cache

    in_maps = [_in_map_for_core(c, x, Wq, bq, Wk, Wv, Wo, slopes)
               for c in range(NCORES)]
    res = run_bass_kernel_spmd(nc, in_maps, core_ids=list(range(NCORES)))
    global LAST_RESULT
    LAST_RESULT = res

    # bk shifts every score in a row i by q_i . bk (constant over j) -> cancels
    # in softmax. bv/bo are linear post-attention terms, folded here exactly.
    bias_term = (bv @ Wo.T + bo)[None, :]
    full = np.zeros((B, S, HID), np.float32)
    for b in range(B):
        acc = np.zeros((S, HID), np.float32)
        for g in range(4):
            acc += np.asarray(res.results[b * 4 + g]["out"]).astype(np.float32)
        full[b] = acc + bias_term
    return full

